# revision 1
# baseline (speedup 1.0000x reference)
"""GAT layer (nn_CustomGATLayer) on 8 Trainium2 NeuronCores.

Strategy (per sharding hint): shard rows of the NxN attention matrix across
8 cores; each core owns N/8=1024 query nodes and holds Wh of all N key nodes
replicated.  Per core, scores are computed directly in transposed [key j,
query i] layout so the attention @ Wh matmul needs no on-device transposes:

  q[j,i]  = madj[j,i] + s2[j] + s1[i]          (one DVE scalar_tensor_tensor)
  r[j,i]  = leaky_relu(q)                      (ACT Prelu, alpha fused; some
                                                chunks on DVE (q*0.2) max q)
  p[j,i]  = exp(r)                             (ACT Exp -> float32r)
  acc[i,:] += p[:,iblk].T @ Wh                 (PE, fp32r, 2 query blocks per
                                                PSUM bank)
  den[i]  += p[:,iblk].T @ [1,1]               (PE, packed 1-bank accumulator)
  out[i,f] = acc[i,f] / den[i]

madj is a host-prepared additive mask in bf16: 0 where the (self-loop added)
adjacency is nonzero, -512 elsewhere, so exp(leaky_relu(t-512)) ~ e^-100 = 0,
matching the reference's hard masking.  Inputs are rolled per-core so every
core runs an identical program (core c's own rows sit first in its local node
order; sums over keys are permutation invariant).  Wh production (phase 1)
and score/accumulate work are emitted interleaved per 1024-node segment so
DMA, PE, ACT and DVE overlap from the start.
"""
import numpy as np
import ml_dtypes
from contextlib import ExitStack

import concourse.bacc as bacc
import concourse.mybir as mybir
import concourse.tile as tile
from concourse.bass_utils import run_bass_kernel_spmd

F32 = mybir.dt.float32
F32R = mybir.dt.float32r
BF16 = mybir.dt.bfloat16
AF = mybir.ActivationFunctionType
ALU = mybir.AluOpType

N = 8192
F = 256
NCORES = 8
R = N // NCORES          # 1024 query rows per core
CH = N // 128            # 64 key chunks of 128
IB = R // 128            # 8 query blocks of 128
SEG = 8                  # phase-1 segments (1024 nodes each)
NBS = CH // SEG          # key chunks per segment
MB = 4                   # key chunks per madj DMA batch
ALPHA = 0.2
BIG = 512.0
# chunk-pairs whose leaky-relu runs on DVE instead of ACT (load balancing)
DVE_LRELU_PAIRS = frozenset({1, 4, 8, 12, 15, 19, 22, 26, 29})
ABLATE = set()            # {"no_den", "no_acc", "no_stt", "no_act"} for bench


def _build(repeat=1):
    nc = bacc.Bacc("TRN2", target_bir_lowering=False, debug=False)
    xT = nc.dram_tensor("xT", [F, N], F32, kind="ExternalInput").ap()
    W = nc.dram_tensor("W", [F, F], F32, kind="ExternalInput").ap()
    WT = nc.dram_tensor("WT", [F, F], F32, kind="ExternalInput").ap()
    a12 = nc.dram_tensor("a12", [F, 2], F32, kind="ExternalInput").ap()
    madjT = nc.dram_tensor("madjT", [N, R], BF16, kind="ExternalInput").ap()
    out = nc.dram_tensor("out", [R, F], F32, kind="ExternalOutput").ap()
    s1d = nc.dram_tensor("s1d", [R], F32).ap()  # bounce for s1 broadcast

    with tile.TileContext(nc) as tc, ExitStack() as ctx:
        persist = ctx.enter_context(tc.tile_pool(name="persist", bufs=1))
        whe = persist.tile([128, CH * F], F32R, tag="whe")
        s1b = persist.tile([128, R], F32, tag="s1b")              # s1 bcast
        s12sb = persist.tile([128, CH * 2], F32, tag="s12sb")     # (s1,s2)/chunk
        w0 = persist.tile([128, F], F32, tag="w0")
        w1 = persist.tile([128, F], F32, tag="w1")
        wr0 = persist.tile([128, F], F32R, tag="wr0")
        wr1 = persist.tile([128, F], F32R, tag="wr1")
        wt0 = persist.tile([128, F], F32, tag="wt0")
        wt1 = persist.tile([128, F], F32, tag="wt1")
        a12t = persist.tile([128, 2, 2], F32, tag="a12t")
        va0 = persist.tile([128, 2], F32, tag="va0")
        va1 = persist.tile([128, 2], F32, tag="va1")
        ones = persist.tile([128, 2], F32, tag="ones")
        onesr = persist.tile([128, 2], F32R, tag="onesr")
        zf = persist.tile([128, 2 * F], F32, tag="zf")
        zr = persist.tile([128, 2 * F], F32R, tag="zr")
        s1row = persist.tile([1, R], F32, tag="s1row")

        nc.sync.dma_start(w0[:], W[0:128, :])
        nc.sync.dma_start(w1[:], W[128:256, :])
        nc.sync.dma_start(wt0[:], WT[0:128, :])
        nc.sync.dma_start(wt1[:], WT[128:256, :])
        nc.sync.dma_start(a12t[:, 0, :], a12[0:128, :])
        nc.sync.dma_start(a12t[:, 1, :], a12[128:256, :])
        nc.vector.tensor_copy(wr0[:], w0[:])
        nc.vector.tensor_copy(wr1[:], w1[:])
        nc.vector.memset(ones[:], 1.0)
        nc.scalar.copy(onesr[:], ones[:])
        nc.vector.memset(zf[:], 0.0)
        nc.scalar.copy(zr[:], zf[:])

        # PSUM budget (8 banks): 4x acc pairs + 1 den + 2 whps + 1 s12/va
        psum = ctx.enter_context(tc.tile_pool(name="psum", bufs=1, space="PSUM"))
        accs = [psum.tile([128, 2 * F], F32, tag=f"acc{i}", name=f"acc{i}")
                for i in range(IB // 2)]
        den = psum.tile([128, 2 * IB], F32, tag="den")
        s12ps = psum.tile([128, CH * 2 + 2], F32, tag="s12ps")
        vps = s12ps[:, CH * 2:CH * 2 + 2]
        whps_pool = ctx.enter_context(
            tc.tile_pool(name="whps", bufs=2, space="PSUM"))

        xpool = ctx.enter_context(tc.tile_pool(name="xstage", bufs=2))
        mpool = ctx.enter_context(tc.tile_pool(name="madj", bufs=3))
        qpool = ctx.enter_context(tc.tile_pool(name="q", bufs=2))
        rpool = ctx.enter_context(tc.tile_pool(name="r", bufs=2))
        ppool = ctx.enter_context(tc.tile_pool(name="p", bufs=2))
        opool = ctx.enter_context(tc.tile_pool(name="o", bufs=2))
        rcpool = ctx.enter_context(tc.tile_pool(name="rc", bufs=2))

        # va = W @ a  (lhsT = W^T); fp32  (re-emitted per timing repeat)
        for _rep in range(repeat):
          wts = (wt0, wt1)
          for kb, va in enumerate((va0, va1)):
            for fc in range(2):
                nc.tensor.matmul(vps, wts[fc][:, kb * 128:(kb + 1) * 128],
                                 a12t[:, fc, :], start=(fc == 0), stop=(fc == 1))
            nc.scalar.copy(va[:], vps)

          for t_acc in accs:
            nc.tensor.matmul(t_acc[:], wr0[:, 0:128], zr[:],
                             start=True, stop=False, skip_group_check=True)
          nc.tensor.matmul(den[:], wr0[:, 0:128], zr[:, 0:2 * IB],
                           start=True, stop=False, skip_group_check=True)

          s12v = s12sb[:].rearrange("p (c t) -> p c t", t=2)
          madj_tiles = {}

          def phase1_segment(s):
            """Load xT segment, round to f32r, compute s12 + Wh chunks."""
            lo = s * R
            xk0 = xpool.tile([128, R], F32, tag="xk0", name="xk0")
            nc.sync.dma_start(xk0[:], xT[0:128, lo:lo + R])
            xk1 = xpool.tile([128, R], F32, tag="xk1", name="xk1")
            nc.sync.dma_start(xk1[:], xT[128:256, lo:lo + R])
            xrs = xpool.tile([128, 2, R], F32R, tag="xrs", name="xrs")
            nc.vector.tensor_copy(xrs[:, 0, :], xk0[:])
            nc.vector.tensor_copy(xrs[:, 1, :], xk1[:])
            for j in range(NBS):
                nb = s * NBS + j
                c0 = j * 128
                nc.tensor.matmul(s12ps[:, nb * 2:nb * 2 + 2],
                                 xk0[:, c0:c0 + 128], va0[:],
                                 start=True, stop=False)
                nc.tensor.matmul(s12ps[:, nb * 2:nb * 2 + 2],
                                 xk1[:, c0:c0 + 128], va1[:],
                                 start=False, stop=True)
                whps = whps_pool.tile([128, F], F32, tag="whps", name="whps")
                nc.tensor.matmul(whps[:], xrs[:, 0, c0:c0 + 128],
                                 wr0[:], start=True, stop=False)
                nc.tensor.matmul(whps[:], xrs[:, 1, c0:c0 + 128],
                                 wr1[:], start=False, stop=True)
                dst = whe[:, nb * F:(nb + 1) * F]
                if nb % 2 == 0:
                    nc.scalar.copy(dst, whps[:])
                else:
                    nc.vector.tensor_copy(dst, whps[:])
            # stage s2 (and s1) of this segment to SBUF
            nc.vector.tensor_copy(s12sb[:, lo // 64:lo // 64 + 2 * NBS],
                                  s12ps[:, lo // 64:lo // 64 + 2 * NBS])

          def madj_load(g):
            mt = mpool.tile([128, MB, R], BF16, tag="madj", name="madj")
            src = madjT.rearrange("(c p) r -> p c r", p=128)[:, g * MB:(g + 1) * MB, :]
            nc.sync.dma_start(mt[:], src)
            madj_tiles[g] = mt

          def main_pair(g):
            c0 = 2 * g
            q = qpool.tile([128, 2, R], F32, tag="q", name="q")
            for h in range(2):
                c = c0 + h
                if c % MB == 0:
                    madj_load(c // MB)
                mt = madj_tiles[c // MB]
                nc.vector.scalar_tensor_tensor(q[:, h, :], mt[:, c % MB, :],
                                               s12v[:, c, 1:2], s1b[:],
                                               op0=ALU.add, op1=ALU.add)
            r = rpool.tile([128, 2, R], F32, tag="r", name="r")
            if g in DVE_LRELU_PAIRS and "all_act" not in ABLATE:
                for h in range(2):
                    nc.vector.scalar_tensor_tensor(r[:, h, :], q[:, h, :],
                                                   ALPHA, q[:, h, :],
                                                   op0=ALU.mult, op1=ALU.max)
            else:
                nc.scalar.activation(r[:].rearrange("p a b -> p (a b)"),
                                     q[:].rearrange("p a b -> p (a b)"),
                                     AF.Prelu, bias=0.0, scale=1.0, alpha=ALPHA)
            p = ppool.tile([128, 2, R], F32R, tag="p", name="p")
            nc.scalar.activation(p[:].rearrange("p a b -> p (a b)"),
                                 r[:].rearrange("p a b -> p (a b)"), AF.Exp)
            for h in range(2):
                c = c0 + h
                for ib in range(IB):
                    lhsT = p[:, h, ib * 128:(ib + 1) * 128]
                    if "no_acc" not in ABLATE:
                        nc.tensor.matmul(
                            accs[ib // 2][:, (ib % 2) * F:(ib % 2 + 1) * F],
                            lhsT, whe[:, c * F:(c + 1) * F],
                            start=False, stop=(c == CH - 1),
                            skip_group_check=True)
                    if "no_den" not in ABLATE:
                        nc.tensor.matmul(den[:, 2 * ib:2 * ib + 2], lhsT,
                                         onesr[:],
                                         start=False, stop=(c == CH - 1),
                                         skip_group_check=True)

          # segment 0 first; s1 broadcast depends only on it
          phase1_segment(0)
          nc.sync.dma_start(s1d.rearrange("(c p) -> p c", p=128), s12v[:, 0:IB, 0])
          nc.sync.dma_start(s1row[:], s1d.rearrange("(o r) -> o r", o=1))
          nc.gpsimd.partition_broadcast(s1b[:], s1row[:])

          # interleave: emit phase-1 segment s, then main pairs of segment s-1
          for s in range(1, SEG):
            phase1_segment(s)
            for g in range((s - 1) * NBS // 2, s * NBS // 2):
                main_pair(g)
          for g in range((SEG - 1) * NBS // 2, CH // 2):
            main_pair(g)

          # normalize + store
          for ib in range(IB):
            rec = rcpool.tile([128, 1], F32, tag="rec", name="rec")
            nc.vector.reciprocal(rec[:], den[:, 2 * ib:2 * ib + 1])
            ot = opool.tile([128, F], F32, tag="ot", name="ot")
            nc.vector.tensor_scalar_mul(
                ot[:], accs[ib // 2][:, (ib % 2) * F:(ib % 2 + 1) * F], rec[:])
            nc.sync.dma_start(out[ib * 128:(ib + 1) * 128, :], ot[:])

    nc.compile()
    return nc


_CACHE = {}


def _get_nc(repeat=1):
    key = f"nc{repeat}"
    if key not in _CACHE:
        _CACHE[key] = _build(repeat)
    return _CACHE[key]


def kernel(adj, x, W, a):
    adj = np.asarray(adj, dtype=np.float32)
    x = np.asarray(x, dtype=np.float32)
    W = np.asarray(W, dtype=np.float32)
    a = np.asarray(a, dtype=np.float32)

    WTc = np.ascontiguousarray(W.T)
    a12 = np.ascontiguousarray(np.stack([a[:F, 0], a[F:, 0]], axis=1))  # [F, 2]
    idx = np.arange(R)

    in_maps = []
    for c in range(NCORES):
        shift = c * R
        xT = np.ascontiguousarray(np.roll(x, -shift, axis=0).T)  # [F, N]
        rows = np.roll(adj[shift:shift + R, :], -shift, axis=1)  # [R, N]
        rows[idx, idx] = 1.0                                     # self loops
        madjT = np.ascontiguousarray(
            np.where(rows > 0, 0.0, -BIG).T.astype(ml_dtypes.bfloat16))
        in_maps.append({"xT": xT, "W": W, "WT": WTc, "a12": a12, "madjT": madjT})

    res = run_bass_kernel_spmd(_get_nc(), in_maps, list(range(NCORES)))
    return np.concatenate([r["out"] for r in res.results], axis=0)


if __name__ == "__main__":
    rng = np.random.default_rng(0)
    adj = (rng.integers(0, 2, (N, N))).astype(np.float32)
    x = rng.normal(size=(N, F)).astype(np.float32)
    W = rng.normal(size=(F, F)).astype(np.float32) * 0.1
    a = rng.normal(size=(2 * F, 1)).astype(np.float32) * 0.1
    out = kernel(adj, x, W, a)
    print(out.shape, out.dtype)



# revision 14
# speedup vs baseline: 2.9265x; 2.9265x over previous
"""GAT layer (nn_CustomGATLayer) on 8 Trainium2 NeuronCores.

Strategy (per sharding hint): shard rows of the NxN attention matrix across
8 cores; each core owns N/8=1024 query nodes and holds Wh of all N key nodes
replicated.  Scores are computed in transposed [key j, query i] layout so the
attention @ Wh matmul needs no on-device transposes.

The per-element exp/leakyrelu chain of the reference is factored away:
  leakyrelu(T) = 0.2*T + 0.8*relu(T)     with  T[j,i] = s1[i] + s2[j]
  exp(lrelu(T)) = e^{0.2 s1} * e^{0.2 s2} * max(e^{0.8 s1} e^{0.8 s2}, 1)
The row factor e^{0.2 s1[i]} cancels between softmax numerator and
denominator; the rank-1 structure means only N+R exps are needed instead of
N*R.  Per element the remaining work is:

  ACT chunks:  t[j,i] = E1b[i] * exp(s2[j])          (ACT Copy, scale=AP)
               p[j,i] = max(t, v[j]) * A[j,i]        (DVE STT, bf16 2x)
  DVE chunks:  t[j,i] = max(E1b[i] * e8s2[j], 1)     (DVE tensor_scalar 4x)
               p[j,i] = t * v[j] * A[j,i]            (DVE STT, bf16 2x)
  acc[i,:] += p[:,iblk].T @ Wh                       (PE bf16: FWL weight load)
  den[i]   += p[:,iblk].T @ [1,1]                    (PE, LDW hidden in acc)
  out[i,f] = acc[i,f] / den[i]

with E1b[i] = exp(0.8*s1[i]) broadcast along partitions, v = exp(0.2*s2),
A = 0/1 adjacency (self-loops added) DMA'd in bf16 [key, query] layout.
s1/s2 are produced for free as two extra rhs columns ([W | W@a1 | W@a2]) of
the Wh matmul, so no separate s12 matmuls (saves 128 weight loads).  All
matmuls are bf16 (fp32r weights forbid fast-weight-load and double LDWEIGHTS
cost).  Wh PSUM->SBUF copies run on GPSIMD to keep ACT/DVE free.  Inputs are
rolled per-core so every core runs an identical program.
"""
import numpy as np
import ml_dtypes
from contextlib import ExitStack

import concourse.bacc as bacc
import concourse.mybir as mybir
import concourse.tile as tile
from concourse.bass_utils import run_bass_kernel_spmd

F32 = mybir.dt.float32
BF16 = mybir.dt.bfloat16
AF = mybir.ActivationFunctionType
ALU = mybir.AluOpType

N = 8192
F = 256
NCORES = 8
R = N // NCORES          # 1024 query rows per core
CH = N // 128            # 64 key chunks of 128
IB = R // 128            # 8 query blocks of 128
SEG = 8                  # phase-1 segments (1024 nodes each)
NBS = CH // SEG          # key chunks per segment
MB = 4                   # key chunks per adjacency DMA batch
FE = F + 2               # whe row: 256 Wh cols + s1 + s2
ALPHA = 0.2
# chunks whose mask-multiply STT runs on GPSIMD instead of DVE (balancing)
GP_STT = frozenset(c for c in range(CH) if c % 4 == 3)


def _build(repeat=1):
    nc = bacc.Bacc("TRN2", target_bir_lowering=False, debug=False)
    xT = nc.dram_tensor("xT", [F, N], BF16, kind="ExternalInput").ap()
    W = nc.dram_tensor("W", [F, F], BF16, kind="ExternalInput").ap()
    WT = nc.dram_tensor("WT", [F, F], BF16, kind="ExternalInput").ap()
    a12 = nc.dram_tensor("a12", [F, 2], BF16, kind="ExternalInput").ap()
    adjT = nc.dram_tensor("adjT", [N, R], BF16, kind="ExternalInput").ap()
    out = nc.dram_tensor("out", [R, F], F32, kind="ExternalOutput").ap()
    e1d = nc.dram_tensor("e1d", [R], F32).ap()  # bounce for E1 broadcast

    with tile.TileContext(nc) as tc, ExitStack() as ctx:
        persist = ctx.enter_context(tc.tile_pool(name="persist", bufs=1))
        whe = persist.tile([128, CH, FE], BF16, tag="whe")
        E1b = persist.tile([128, R], BF16, tag="E1b")
        e1f = persist.tile([128, R], F32, tag="e1f")
        w0x = persist.tile([128, FE], BF16, tag="w0x")
        w1x = persist.tile([128, FE], BF16, tag="w1x")
        wt0 = persist.tile([128, F], BF16, tag="wt0")
        wt1 = persist.tile([128, F], BF16, tag="wt1")
        a12t = persist.tile([128, 2, 2], BF16, tag="a12t")
        ones = persist.tile([128, 2], BF16, tag="ones")
        zr = persist.tile([128, 2 * F], BF16, tag="zr")
        es2 = persist.tile([128, CH], F32, tag="es2")    # exp(s2)
        vs2 = persist.tile([128, CH], F32, tag="vs2")    # exp(0.2 s2)
        e1t = persist.tile([128, IB], F32, tag="e1t")    # exp(0.8 s1) own
        e1row = persist.tile([1, R], F32, tag="e1row")

        nc.sync.dma_start(w0x[:, 0:F], W[0:128, :])
        nc.sync.dma_start(w1x[:, 0:F], W[128:256, :])
        nc.sync.dma_start(wt0[:], WT[0:128, :])
        nc.sync.dma_start(wt1[:], WT[128:256, :])
        nc.sync.dma_start(a12t[:, 0, :], a12[0:128, :])
        nc.sync.dma_start(a12t[:, 1, :], a12[128:256, :])
        nc.vector.memset(ones[:], 1.0)
        nc.vector.memset(zr[:], 0.0)

        # PSUM budget (8 banks): 4x acc pairs + 1 den + 2 whps + 1 vps
        psum = ctx.enter_context(tc.tile_pool(name="psum", bufs=1, space="PSUM"))
        accs = [psum.tile([128, 2 * F], F32, tag=f"acc{i}", name=f"acc{i}")
                for i in range(IB // 2)]
        den = psum.tile([128, 2 * IB], F32, tag="den")
        vps = psum.tile([128, 2], F32, tag="vps")
        whps_pool = ctx.enter_context(
            tc.tile_pool(name="whps", bufs=2, space="PSUM"))

        xpool = ctx.enter_context(tc.tile_pool(name="xstage", bufs=2))
        mpool = ctx.enter_context(tc.tile_pool(name="madj", bufs=3))
        tpool = ctx.enter_context(tc.tile_pool(name="t", bufs=3))
        ppool = ctx.enter_context(tc.tile_pool(name="p", bufs=3))
        opool = ctx.enter_context(tc.tile_pool(name="o", bufs=2))
        rcpool = ctx.enter_context(tc.tile_pool(name="rc", bufs=2))

        for _rep in range(repeat):
          # va = W @ a  (lhsT = W^T) into w0x/w1x cols 256:258
          wts = (wt0, wt1)
          for kb, wx in enumerate((w0x, w1x)):
            for fc in range(2):
                nc.tensor.matmul(vps, wts[fc][:, kb * 128:(kb + 1) * 128],
                                 a12t[:, fc, :], start=(fc == 0), stop=(fc == 1))
            nc.vector.tensor_copy(wx[:, F:F + 2], vps)

          # zero-init accumulator banks: a single full-width start=True
          # matmul per bank (a per-iblock start=True would clear the whole
          # bank's has_written bits and drop sibling iblocks' first chunk)
          for t_acc in accs:
            nc.tensor.matmul(t_acc[:], w0x[:, 0:128], zr[:],
                             start=True, stop=False, skip_group_check=True)
          nc.tensor.matmul(den[:], w0x[:, 0:128], zr[:, 0:2 * IB],
                           start=True, stop=False, skip_group_check=True)

          adj_tiles = {}

          def phase1_segment(s):
            """Load xT segment, compute Wh chunks (+ s1,s2 as extra cols)."""
            lo = s * R
            xk0 = xpool.tile([128, R], BF16, tag="xk0", name="xk0")
            nc.sync.dma_start(xk0[:], xT[0:128, lo:lo + R])
            xk1 = xpool.tile([128, R], BF16, tag="xk1", name="xk1")
            nc.sync.dma_start(xk1[:], xT[128:256, lo:lo + R])
            for j in range(NBS):
                nb = s * NBS + j
                c0 = j * 128
                whps = whps_pool.tile([128, FE], F32, tag="whps", name="whps")
                nc.tensor.matmul(whps[:], xk0[:, c0:c0 + 128],
                                 w0x[:], start=True, stop=False)
                nc.tensor.matmul(whps[:], xk1[:, c0:c0 + 128],
                                 w1x[:], start=False, stop=True)
                nc.scalar.copy(whe[:, nb, :], whps[:])
            # batch the per-key scalar exps for this segment
            s2seg = whe[:, s * NBS:(s + 1) * NBS, 257]
            c0 = s * NBS
            nc.scalar.activation(es2[:, c0:c0 + NBS], s2seg, AF.Exp)
            nc.scalar.activation(vs2[:, c0:c0 + NBS], s2seg, AF.Exp, scale=0.2)

          def adj_load(g):
            mt = mpool.tile([128, MB, R], BF16, tag="adj", name="adj")
            src = adjT.rearrange("(c p) r -> p c r", p=128)[:, g * MB:(g + 1) * MB, :]
            nc.sync.dma_start(mt[:], src)
            adj_tiles[g] = mt

          def main_pair(g):
            c0 = 2 * g
            t = tpool.tile([128, 2, R], BF16, tag="t", name="t")
            p = ppool.tile([128, 2, R], BF16, tag="p", name="p")
            for h in range(2):
                c = c0 + h
                if c % MB == 0:
                    adj_load(c // MB)
                At = adj_tiles[c // MB][:, c % MB, :]
                # t = max(E1b * exp(s2_j), exp(.2 s2_j));  p = t * A
                nc.vector.tensor_scalar(t[:, h, :], E1b[:],
                                        es2[:, c:c + 1], vs2[:, c:c + 1],
                                        op0=ALU.mult, op1=ALU.max)
                eng = nc.gpsimd if c in GP_STT else nc.vector
                eng.tensor_tensor(p[:, h, :], t[:, h, :], At, op=ALU.mult)
            for h in range(2):
                c = c0 + h
                for ib in range(IB):
                    lhsT = p[:, h, ib * 128:(ib + 1) * 128]
                    nc.tensor.matmul(
                        accs[ib // 2][:, (ib % 2) * F:(ib % 2 + 1) * F],
                        lhsT, whe[:, c, 0:F],
                        start=False, stop=(c == CH - 1),
                        skip_group_check=True)
                    nc.tensor.matmul(den[:, 2 * ib:2 * ib + 2], lhsT,
                                     ones[:],
                                     start=False, stop=(c == CH - 1),
                                     skip_group_check=True)

          # segment 0 first; E1 broadcast depends only on it
          phase1_segment(0)
          nc.scalar.activation(e1t[:], whe[:, 0:IB, 256], AF.Exp, scale=0.8)
          nc.sync.dma_start(e1d.rearrange("(c p) -> p c", p=128), e1t[:])
          nc.sync.dma_start(e1row[:], e1d.rearrange("(o r) -> o r", o=1))
          nc.gpsimd.partition_broadcast(e1f[:], e1row[:])
          nc.vector.tensor_copy(E1b[:], e1f[:])

          # interleave: emit phase-1 segment s, then main pairs of segment s-1
          for s in range(1, SEG):
            phase1_segment(s)
            for g in range((s - 1) * NBS // 2, s * NBS // 2):
                main_pair(g)
          for g in range((SEG - 1) * NBS // 2, CH // 2):
            main_pair(g)

          # normalize + store
          for ib in range(IB):
            rec = rcpool.tile([128, 1], F32, tag="rec", name="rec")
            nc.vector.reciprocal(rec[:], den[:, 2 * ib:2 * ib + 1])
            ot = opool.tile([128, F], F32, tag="ot", name="ot")
            nc.vector.tensor_scalar_mul(
                ot[:], accs[ib // 2][:, (ib % 2) * F:(ib % 2 + 1) * F], rec[:])
            nc.sync.dma_start(out[ib * 128:(ib + 1) * 128, :], ot[:])

    nc.compile()
    return nc


_CACHE = {}


def _get_nc(repeat=1):
    key = f"nc{repeat}"
    if key not in _CACHE:
        _CACHE[key] = _build(repeat)
    return _CACHE[key]


def make_in_maps(adj, x, W, a):
    adj = np.asarray(adj, dtype=np.float32)
    x = np.asarray(x, dtype=np.float32)
    W = np.asarray(W, dtype=np.float32)
    a = np.asarray(a, dtype=np.float32)

    Wb = W.astype(ml_dtypes.bfloat16)
    WTb = np.ascontiguousarray(W.T).astype(ml_dtypes.bfloat16)
    a12 = np.ascontiguousarray(
        np.stack([a[:F, 0], a[F:, 0]], axis=1)).astype(ml_dtypes.bfloat16)
    idx = np.arange(R)

    in_maps = []
    for c in range(NCORES):
        shift = c * R
        xTb = np.ascontiguousarray(
            np.roll(x, -shift, axis=0).T).astype(ml_dtypes.bfloat16)
        rows = np.roll(adj[shift:shift + R, :], -shift, axis=1)  # [R, N]
        rows[idx, idx] = 1.0                                     # self loops
        adjTb = np.ascontiguousarray(
            (rows > 0).T.astype(ml_dtypes.bfloat16))             # [N, R] 0/1
        in_maps.append({"xT": xTb, "W": Wb, "WT": WTb, "a12": a12,
                        "adjT": adjTb})
    return in_maps


def kernel(adj, x, W, a):
    in_maps = make_in_maps(adj, x, W, a)
    res = run_bass_kernel_spmd(_get_nc(), in_maps, list(range(NCORES)))
    return np.concatenate([r["out"] for r in res.results], axis=0)


if __name__ == "__main__":
    rng = np.random.default_rng(0)
    adj = (rng.integers(0, 2, (N, N))).astype(np.float32)
    x = rng.normal(size=(N, F)).astype(np.float32)
    W = rng.normal(size=(F, F)).astype(np.float32) * 0.1
    a = rng.normal(size=(2 * F, 1)).astype(np.float32) * 0.1
    out = kernel(adj, x, W, a)
    print(out.shape, out.dtype)


# revision 16
# speedup vs baseline: 6.8290x; 2.3335x over previous
"""GAT layer (nn_CustomGATLayer) on 8 Trainium2 NeuronCores.

Strategy (per sharding hint): shard rows of the NxN attention matrix across
8 cores; each core owns N/8=1024 query nodes and holds Wh of all N key nodes
replicated.  Scores are computed in transposed [key j, query i] layout so the
attention @ Wh matmul needs no on-device transposes.

The per-element exp/leakyrelu chain of the reference is factored away:
  leakyrelu(T) = 0.2*T + 0.8*relu(T)     with  T[j,i] = s1[i] + s2[j]
  exp(lrelu(T)) = e^{0.2 s1} * e^{0.2 s2} * max(e^{0.8 s1} e^{0.8 s2}, 1)
The row factor e^{0.2 s1[i]} cancels between softmax numerator and
denominator; the rank-1 structure means only N+R exps are needed instead of
N*R.  Per element the remaining work is:

  ACT chunks:  t[j,i] = E1b[i] * exp(s2[j])          (ACT Copy, scale=AP)
               p[j,i] = max(t, v[j]) * A[j,i]        (DVE STT, bf16 2x)
  DVE chunks:  t[j,i] = max(E1b[i] * e8s2[j], 1)     (DVE tensor_scalar 4x)
               p[j,i] = t * v[j] * A[j,i]            (DVE STT, bf16 2x)
  acc[i,:] += p[:,iblk].T @ Wh                       (PE bf16: FWL weight load)
  den[i]   += p[:,iblk].T @ [1,1]                    (PE, LDW hidden in acc)
  out[i,f] = acc[i,f] / den[i]

with E1b[i] = exp(0.8*s1[i]) broadcast along partitions, v = exp(0.2*s2),
A = 0/1 adjacency (self-loops added) DMA'd in bf16 [key, query] layout.
s1/s2 are produced for free as two extra rhs columns ([W | W@a1 | W@a2]) of
the Wh matmul, so no separate s12 matmuls (saves 128 weight loads).  All
matmuls are bf16 (fp32r weights forbid fast-weight-load and double LDWEIGHTS
cost).  Wh PSUM->SBUF copies run on GPSIMD to keep ACT/DVE free.  Inputs are
rolled per-core so every core runs an identical program.
"""
import numpy as np
import ml_dtypes
from contextlib import ExitStack

import concourse.bacc as bacc
import concourse.mybir as mybir
import concourse.tile as tile
from concourse.bass_utils import run_bass_kernel_spmd

F32 = mybir.dt.float32
BF16 = mybir.dt.bfloat16
AF = mybir.ActivationFunctionType
ALU = mybir.AluOpType

N = 8192
F = 256
NCORES = 8
R = N // NCORES          # 1024 query rows per core
CH = N // 128            # 64 key chunks of 128
IB = R // 128            # 8 query blocks of 128
SEG = 8                  # phase-1 segments (1024 nodes each)
NBS = CH // SEG          # key chunks per segment
MB = 4                   # key chunks per adjacency DMA batch
FE = F + 2               # whe row: 256 Wh cols + s1 + s2
ALPHA = 0.2
# chunks whose mask-multiply STT runs on GPSIMD instead of DVE (balancing)
GP_STT = frozenset(c for c in range(CH) if c % 4 == 3)


def _build(repeat=1):
    nc = bacc.Bacc("TRN2", target_bir_lowering=False, debug=False)
    xT = nc.dram_tensor("xT", [F, N], BF16, kind="ExternalInput").ap()
    W = nc.dram_tensor("W", [F, F], BF16, kind="ExternalInput").ap()
    WT = nc.dram_tensor("WT", [F, F], BF16, kind="ExternalInput").ap()
    a12 = nc.dram_tensor("a12", [F, 2], BF16, kind="ExternalInput").ap()
    adjT = nc.dram_tensor("adjT", [N, R], BF16, kind="ExternalInput").ap()
    out = nc.dram_tensor("out", [R, F], F32, kind="ExternalOutput").ap()
    e1d = nc.dram_tensor("e1d", [R], F32).ap()  # bounce for E1 broadcast

    with tile.TileContext(nc) as tc, ExitStack() as ctx:
        persist = ctx.enter_context(tc.tile_pool(name="persist", bufs=1))
        whe = persist.tile([128, CH, FE], BF16, tag="whe")
        E1b = persist.tile([128, R], BF16, tag="E1b")
        e1f = persist.tile([128, R], F32, tag="e1f")
        w0x = persist.tile([128, FE], BF16, tag="w0x")
        w1x = persist.tile([128, FE], BF16, tag="w1x")
        wt0 = persist.tile([128, F], BF16, tag="wt0")
        wt1 = persist.tile([128, F], BF16, tag="wt1")
        a12t = persist.tile([128, 2, 2], BF16, tag="a12t")
        ones = persist.tile([128, 2], BF16, tag="ones")
        zr = persist.tile([128, 2 * F], BF16, tag="zr")
        es2 = persist.tile([128, CH], F32, tag="es2")    # exp(s2)
        vs2 = persist.tile([128, CH], F32, tag="vs2")    # exp(0.2 s2)
        e1t = persist.tile([128, IB], F32, tag="e1t")    # exp(0.8 s1) own
        e1row = persist.tile([1, R], F32, tag="e1row")

        nc.sync.dma_start(w0x[:, 0:F], W[0:128, :])
        nc.sync.dma_start(w1x[:, 0:F], W[128:256, :])
        nc.sync.dma_start(wt0[:], WT[0:128, :])
        nc.sync.dma_start(wt1[:], WT[128:256, :])
        nc.sync.dma_start(a12t[:, 0, :], a12[0:128, :])
        nc.sync.dma_start(a12t[:, 1, :], a12[128:256, :])
        nc.vector.memset(ones[:], 1.0)
        nc.vector.memset(zr[:], 0.0)

        # PSUM budget (8 banks): 4x acc pairs + 1 den + 2 whps + 1 vps
        psum = ctx.enter_context(tc.tile_pool(name="psum", bufs=1, space="PSUM"))
        accs = [psum.tile([128, 2 * F], F32, tag=f"acc{i}", name=f"acc{i}")
                for i in range(IB // 2)]
        den = psum.tile([128, 2 * IB], F32, tag="den")
        vps = psum.tile([128, 2], F32, tag="vps")
        whps_pool = ctx.enter_context(
            tc.tile_pool(name="whps", bufs=2, space="PSUM"))

        xpool = ctx.enter_context(tc.tile_pool(name="xstage", bufs=2))
        mpool = ctx.enter_context(tc.tile_pool(name="madj", bufs=3))
        tpool = ctx.enter_context(tc.tile_pool(name="t", bufs=3))
        ppool = ctx.enter_context(tc.tile_pool(name="p", bufs=3))
        opool = ctx.enter_context(tc.tile_pool(name="o", bufs=2))
        rcpool = ctx.enter_context(tc.tile_pool(name="rc", bufs=2))

        for _rep in range(repeat):
          # va = W @ a  (lhsT = W^T) into w0x/w1x cols 256:258
          wts = (wt0, wt1)
          for kb, wx in enumerate((w0x, w1x)):
            for fc in range(2):
                nc.tensor.matmul(vps, wts[fc][:, kb * 128:(kb + 1) * 128],
                                 a12t[:, fc, :], start=(fc == 0), stop=(fc == 1))
            nc.vector.tensor_copy(wx[:, F:F + 2], vps)

          # zero-init accumulator banks: a single full-width start=True
          # matmul per bank (a per-iblock start=True would clear the whole
          # bank's has_written bits and drop sibling iblocks' first chunk)
          for t_acc in accs:
            nc.tensor.matmul(t_acc[:], w0x[:, 0:128], zr[:],
                             start=True, stop=False, skip_group_check=True)
          nc.tensor.matmul(den[:], w0x[:, 0:128], zr[:, 0:2 * IB],
                           start=True, stop=False, skip_group_check=True)

          adj_tiles = {}

          def phase1_segment(s):
            """Load xT segment, compute Wh chunks (+ s1,s2 as extra cols)."""
            lo = s * R
            xk0 = xpool.tile([128, R], BF16, tag="xk0", name="xk0")
            nc.sync.dma_start(xk0[:], xT[0:128, lo:lo + R])
            xk1 = xpool.tile([128, R], BF16, tag="xk1", name="xk1")
            nc.sync.dma_start(xk1[:], xT[128:256, lo:lo + R])
            for j in range(NBS):
                nb = s * NBS + j
                c0 = j * 128
                whps = whps_pool.tile([128, FE], F32, tag="whps", name="whps")
                nc.tensor.matmul(whps[:], xk0[:, c0:c0 + 128],
                                 w0x[:], start=True, stop=False)
                nc.tensor.matmul(whps[:], xk1[:, c0:c0 + 128],
                                 w1x[:], start=False, stop=True)
                nc.scalar.copy(whe[:, nb, :], whps[:])
            # batch the per-key scalar exps for this segment
            s2seg = whe[:, s * NBS:(s + 1) * NBS, 257]
            c0 = s * NBS
            nc.scalar.activation(es2[:, c0:c0 + NBS], s2seg, AF.Exp)
            nc.scalar.activation(vs2[:, c0:c0 + NBS], s2seg, AF.Exp, scale=0.2)

          def adj_load(g):
            mt = mpool.tile([128, MB, R], BF16, tag="adj", name="adj")
            src = adjT.rearrange("(c p) r -> p c r", p=128)[:, g * MB:(g + 1) * MB, :]
            nc.sync.dma_start(mt[:], src)
            adj_tiles[g] = mt

          def main_pair(g):
            c0 = 2 * g
            t = tpool.tile([128, 2, R], BF16, tag="t", name="t")
            p = ppool.tile([128, 2, R], BF16, tag="p", name="p")
            for h in range(2):
                c = c0 + h
                if c % MB == 0:
                    adj_load(c // MB)
                At = adj_tiles[c // MB][:, c % MB, :]
                # t = max(E1b * exp(s2_j), exp(.2 s2_j));  p = t * A
                nc.vector.tensor_scalar(t[:, h, :], E1b[:],
                                        es2[:, c:c + 1], vs2[:, c:c + 1],
                                        op0=ALU.mult, op1=ALU.max)
                eng = nc.gpsimd if c in GP_STT else nc.vector
                eng.tensor_tensor(p[:, h, :], t[:, h, :], At, op=ALU.mult)
            for h in range(2):
                c = c0 + h
                for ib in range(IB):
                    lhsT = p[:, h, ib * 128:(ib + 1) * 128]
                    nc.tensor.matmul(
                        accs[ib // 2][:, (ib % 2) * F:(ib % 2 + 1) * F],
                        lhsT, whe[:, c, 0:F],
                        start=False, stop=(c == CH - 1),
                        skip_group_check=True)
                    nc.tensor.matmul(den[:, 2 * ib:2 * ib + 2], lhsT,
                                     ones[:],
                                     start=False, stop=(c == CH - 1),
                                     skip_group_check=True)

          # segment 0 first; E1 broadcast depends only on it
          phase1_segment(0)
          nc.scalar.activation(e1t[:], whe[:, 0:IB, 256], AF.Exp, scale=0.8)
          nc.sync.dma_start(e1d.rearrange("(c p) -> p c", p=128), e1t[:])
          nc.sync.dma_start(e1row[:], e1d.rearrange("(o r) -> o r", o=1))
          nc.gpsimd.partition_broadcast(e1f[:], e1row[:])
          nc.vector.tensor_copy(E1b[:], e1f[:])

          # interleave: emit phase-1 segment s, then main pairs of segment s-1
          for s in range(1, SEG):
            phase1_segment(s)
            for g in range((s - 1) * NBS // 2, s * NBS // 2):
                main_pair(g)
          for g in range((SEG - 1) * NBS // 2, CH // 2):
            main_pair(g)

          # normalize + store
          for ib in range(IB):
            rec = rcpool.tile([128, 1], F32, tag="rec", name="rec")
            nc.vector.reciprocal(rec[:], den[:, 2 * ib:2 * ib + 1])
            ot = opool.tile([128, F], F32, tag="ot", name="ot")
            nc.vector.tensor_scalar_mul(
                ot[:], accs[ib // 2][:, (ib % 2) * F:(ib % 2 + 1) * F], rec[:])
            nc.sync.dma_start(out[ib * 128:(ib + 1) * 128, :], ot[:])

    nc.compile()
    return nc


_CACHE = {}


def _get_nc(repeat=1):
    key = f"nc{repeat}"
    if key not in _CACHE:
        _CACHE[key] = _build(repeat)
    return _CACHE[key]


def make_in_maps(adj, x, W, a):
    adj = np.asarray(adj, dtype=np.float32)
    x = np.asarray(x, dtype=np.float32)
    W = np.asarray(W, dtype=np.float32)
    a = np.asarray(a, dtype=np.float32)

    Wb = W.astype(ml_dtypes.bfloat16)
    WTb = np.ascontiguousarray(W.T).astype(ml_dtypes.bfloat16)
    a12 = np.ascontiguousarray(
        np.stack([a[:F, 0], a[F:, 0]], axis=1)).astype(ml_dtypes.bfloat16)
    idx = np.arange(R)

    in_maps = []
    for c in range(NCORES):
        shift = c * R
        xTb = np.ascontiguousarray(
            np.roll(x, -shift, axis=0).T).astype(ml_dtypes.bfloat16)
        rows = np.roll(adj[shift:shift + R, :], -shift, axis=1)  # [R, N]
        rows[idx, idx] = 1.0                                     # self loops
        adjTb = np.ascontiguousarray(
            (rows > 0).T.astype(ml_dtypes.bfloat16))             # [N, R] 0/1
        in_maps.append({"xT": xTb, "W": Wb, "WT": WTb, "a12": a12,
                        "adjT": adjTb})
    return in_maps


def kernel(adj, x, W, a):
    in_maps = make_in_maps(adj, x, W, a)
    res = run_bass_kernel_spmd(_get_nc(), in_maps, list(range(NCORES)))
    return np.concatenate([r["out"] for r in res.results], axis=0)


if __name__ == "__main__":
    rng = np.random.default_rng(0)
    adj = (rng.integers(0, 2, (N, N))).astype(np.float32)
    x = rng.normal(size=(N, F)).astype(np.float32)
    W = rng.normal(size=(F, F)).astype(np.float32) * 0.1
    a = rng.normal(size=(2 * F, 1)).astype(np.float32) * 0.1
    out = kernel(adj, x, W, a)
    print(out.shape, out.dtype)
